# revision 8
# baseline (speedup 1.0000x reference)
"""ContrastLoss (InfoNCE-style) Trainium2 kernel, data-parallel over batch on 8 cores.

Math (per sample b):
    s[i,j] = (tmap[b,i,j] . qhat[b]) / ||tmap[b,i,j]||        (qhat = normalized pos_query)
    e = exp(s); num = sum(e * pos_mask); den = num + sum(e * neg_mask)
    li = -log(num / (den + EPS)); loss = mean(li over valid samples)

Device design per core (4 samples, 4096 cells each, H=256), fp8 shipments:
  tmap is shipped TWICE in fp8-e4m3 (together = bf16-sized traffic, 8.4 MB):
  - t_h: H-on-partition blocks for the TensorEngine. Per 128-cell block j:
    lhsT = t-block (128 h-half x 128 cells), rhs = qhat half-column ->
    out = PSUM column of per-cell dots, cells on partitions. Both H-halves
    accumulate into the same PSUM column (start/stop). FWL makes the
    LDWEIGHTS+MATMUL stream ~40ns/pair, so all dots cost ~10us of PE.
  - t_c: cells-on-partition layout (cell = 32*p + j) for sum-of-squares:
    fused multiply+reduce per (128,256) sub-tile, split between ScalarE
    activation(Square, accum_out) and DVE scalar_tensor_tensor for balance.
  Epilogue batched core-wide on (128, 4*32) fp32 stat tiles:
  1/||t|| = exp(-0.5*ln(ssq)) on ScalarE, s = dot/||t||, e = exp(s), then
  per-sample masked sums; 128 partial sums per sample go to the host for the
  final tiny reduction (-log, valid masking, mean over 32 samples).
"""

import numpy as np
import ml_dtypes

import concourse.bacc as bacc
import concourse.tile as tile
from concourse import mybir
from concourse.bass_utils import run_bass_kernel_spmd
from concourse.hw_specs import get_activation_tables as _real_gat

_ACT_SET = "natural_log_exp_and_others"  # contains square, ln, exp


def _patched_gat(arch):
    """Force every activation to resolve to the one set containing all our
    functions (square/ln/exp), avoiding per-sample table-set thrashing
    (~2.7us per reload). Indices into act_info.json are preserved."""
    tabs = _real_gat(arch)
    return {k: (v if k == _ACT_SET else set()) for k, v in tabs.items()}


bacc.get_activation_tables = _patched_gat

N_CORES = 8
B, S, H = 32, 64, 256
BS = B // N_CORES          # samples per core
CELLS = S * S              # 4096 cells per sample
NBLK = CELLS // 128        # 32 cell-blocks (also 32 ssq sub-tiles) per sample
EPS = 1e-8
F8 = ml_dtypes.float8_e4m3
BF16 = ml_dtypes.bfloat16

# Of every 16 sum-of-squares sub-tiles, this many run on ScalarE (rest DVE).
# ScalarE accum-activations cost ~480ns (main + accumulator read) vs ~330ns
# for a DVE fused stt, so DVE takes the bigger share.
ACT_SSQ_OF16 = 6

# Calibration knobs (timing experiments only; extra work goes to scratch and
# the kernel output is unchanged). Per SAMPLE counts.
EXTRA_PE_PAIRS = 0   # duplicate LS+MM pairs into a scratch PSUM column
EXTRA_SSQ_DVE = 0    # duplicate DVE fused ssq into a scratch accum column
EXTRA_SSQ_ACT = 0    # duplicate ScalarE accum-squares into a scratch column
EXTRA_DMA = 0        # duplicate th chunk DMAs into a scratch tile

_NC_CACHE = {}


def _build_nc(loop_reps=0):
    """loop_reps=0: straight-line kernel. loop_reps=N>0: wrap the whole body
    in a tc.For_i loop that re-runs it N times (identical data; used only for
    differential wall-clock timing of the device execution)."""
    A = mybir.ActivationFunctionType
    OP = mybir.AluOpType
    dt = mybir.dt

    nc = bacc.Bacc(
        "TRN2",
        target_bir_lowering=False,
        debug=False,
        enable_asserts=False,
        num_devices=N_CORES,
    )

    th_in = nc.dram_tensor("th_in", [BS, 128, 2 * CELLS], dt.float8e4, kind="ExternalInput").ap()
    tc_in = nc.dram_tensor("tc_in", [BS, 128, NBLK * H], dt.float8e4, kind="ExternalInput").ap()
    qh_in = nc.dram_tensor("qh_in", [128, 2 * BS], dt.bfloat16, kind="ExternalInput").ap()
    pm_in = nc.dram_tensor("pm_in", [128, BS * NBLK], dt.float32, kind="ExternalInput").ap()
    nm_in = nc.dram_tensor("nm_in", [128, BS * NBLK], dt.float32, kind="ExternalInput").ap()
    parts = nc.dram_tensor("parts", [128, 2 * BS], dt.float32, kind="ExternalOutput").ap()

    with tile.TileContext(nc) as tc:
        with (
            tc.tile_pool(name="chunks", bufs=3) as chpool,
            tc.tile_pool(name="small", bufs=1) as spool,
            tc.tile_pool(name="stats", bufs=1) as stpool,
            tc.tile_pool(name="psum", bufs=2, space="PSUM") as pspool,
        ):
            qsb = spool.tile([128, 2 * BS], dt.bfloat16, tag="qsb")
            nc.sync.dma_start(out=qsb[:], in_=qh_in[:])
            pmsb = spool.tile([128, BS * NBLK], dt.float32, tag="pmsb")
            nc.sync.dma_start(out=pmsb[:], in_=pm_in[:])
            nmsb = spool.tile([128, BS * NBLK], dt.float32, tag="nmsb")
            nc.sync.dma_start(out=nmsb[:], in_=nm_in[:])

            npart = spool.tile([128, 2 * BS], dt.float32, tag="npart")
            dve_scr = spool.tile([128, H], dt.float8e4, tag="dve_scr")
            act_scr = spool.tile([128, H], dt.float8e4, tag="act_scr")
            msk_scr = spool.tile([128, NBLK], dt.float32, tag="msk_scr")

            import contextlib
            loop_cm = tc.For_i(0, loop_reps, 1) if loop_reps else contextlib.nullcontext()
            with loop_cm:
                _emit_body(nc, tc, spool, stpool, chpool, pspool,
                           th_in, tc_in, qsb, pmsb, nmsb, npart,
                           dve_scr, act_scr, msk_scr, A, OP, dt)

            nc.sync.dma_start(out=parts[:], in_=npart[:])

    nc.compile()
    return nc


def _emit_body(nc, tc, spool, stpool, chpool, pspool, th_in, tc_in, qsb,
               pmsb, nmsb, npart, dve_scr, act_scr, msk_scr, A, OP, dt):
    # Core-wide stat tiles: column NBLK*s + j for sample s, block j.
    dotb = stpool.tile([128, BS * NBLK], dt.float32, tag="dotb")
    ssqb = stpool.tile([128, BS * NBLK], dt.float32, tag="ssqb")
    if EXTRA_SSQ_DVE or EXTRA_SSQ_ACT:
        scrq = stpool.tile([128, 4], dt.float32, tag="scrq")
    gidx = 0
    for s in range(BS):
        th = chpool.tile([128, 2 * CELLS], dt.float8e4, tag="th")
        nc.sync.dma_start(out=th[:], in_=th_in[s])
        tcl = chpool.tile([128, NBLK * H], dt.float8e4, tag="tcl")
        nc.sync.dma_start(out=tcl[:], in_=tc_in[s])
        for e in range(EXTRA_DMA):
            xtr = chpool.tile([128, 2 * CELLS], dt.float8e4, tag="xtr")
            nc.sync.dma_start(out=xtr[:], in_=th_in[s])

        psum_s = pspool.tile([128, NBLK], dt.float32, tag="ps")
        if EXTRA_PE_PAIRS:
            psum_scr = pspool.tile([128, 1], dt.float32, tag="ps_scr")
        else:
            psum_scr = None
        for j in range(NBLK):
            for k in range(2):
                nc.tensor.matmul(
                    psum_s[:, j:j + 1],
                    th[:, k * CELLS + j * 128:k * CELLS + (j + 1) * 128],
                    qsb[:, 2 * s + k:2 * s + k + 1],
                    start=(k == 0),
                    stop=(k == 1),
                )
            for e in range(EXTRA_PE_PAIRS // NBLK):
                nc.tensor.matmul(
                    psum_scr[:, 0:1],
                    th[:, (e % 2) * CELLS + j * 128:(e % 2) * CELLS + (j + 1) * 128],
                    qsb[:, 2 * s:2 * s + 1],
                    start=True,
                    stop=True,
                )
            sub = tcl[:, j * H:(j + 1) * H]
            col = NBLK * s + j
            if gidx % 16 < ACT_SSQ_OF16:
                nc.scalar.activation(
                    act_scr[:], sub, A.Square,
                    accum_out=ssqb[:, col:col + 1],
                )
            else:
                nc.vector.scalar_tensor_tensor(
                    out=dve_scr[:],
                    in0=sub,
                    scalar=0.0,
                    in1=sub,
                    op0=OP.bypass,
                    op1=OP.mult,
                    accum_out=ssqb[:, col:col + 1],
                )
            for e in range(EXTRA_SSQ_DVE // NBLK):
                nc.vector.scalar_tensor_tensor(
                    out=dve_scr[:], in0=sub, scalar=0.0, in1=sub,
                    op0=OP.bypass, op1=OP.mult, accum_out=scrq[:, 0:1],
                )
            for e in range(EXTRA_SSQ_ACT // NBLK):
                nc.scalar.activation(
                    act_scr[:], sub, A.Square, accum_out=scrq[:, 1:2],
                )
            gidx += 1
        nc.vector.tensor_copy(out=dotb[:, NBLK * s:NBLK * (s + 1)], in_=psum_s[:])

    # Batched epilogue on (128, BS*NBLK) fp32 stat tiles.
    lnb = stpool.tile([128, BS * NBLK], dt.float32, tag="lnb")
    nc.scalar.activation(lnb[:], ssqb[:], A.Ln)
    invn = stpool.tile([128, BS * NBLK], dt.float32, tag="invn")
    nc.scalar.activation(invn[:], lnb[:], A.Exp, scale=-0.5)
    sb = stpool.tile([128, BS * NBLK], dt.float32, tag="sb")
    nc.vector.tensor_mul(sb[:], dotb[:], invn[:])
    eb = stpool.tile([128, BS * NBLK], dt.float32, tag="eb")
    nc.scalar.activation(eb[:], sb[:], A.Exp)
    for s in range(BS):
        nc.vector.scalar_tensor_tensor(
            out=msk_scr[:], in0=eb[:, NBLK * s:NBLK * s + NBLK], scalar=0.0,
            in1=pmsb[:, s * NBLK:(s + 1) * NBLK],
            op0=OP.bypass, op1=OP.mult,
            accum_out=npart[:, 2 * s:2 * s + 1],
        )
        nc.vector.scalar_tensor_tensor(
            out=msk_scr[:], in0=eb[:, NBLK * s:NBLK * s + NBLK], scalar=0.0,
            in1=nmsb[:, s * NBLK:(s + 1) * NBLK],
            op0=OP.bypass, op1=OP.mult,
            accum_out=npart[:, 2 * s + 1:2 * s + 2],
        )


def get_nc(loop_reps=0):
    key = ("nc", loop_reps)
    if key not in _NC_CACHE:
        _NC_CACHE[key] = _build_nc(loop_reps)
    return _NC_CACHE[key]


def make_in_maps(pos_query, tmap, mask2d_pos, mask2d_neg):
    pq = np.asarray(pos_query, dtype=np.float32)
    tm = np.asarray(tmap, dtype=np.float32)
    mp = np.asarray(mask2d_pos).astype(bool)
    mn = np.asarray(mask2d_neg).astype(bool)

    qn = np.sqrt(np.sum(pq * pq, axis=-1, keepdims=True, dtype=np.float32))
    qhat = (pq / (qn + np.float32(EPS))).astype(BF16)

    # fp8 cast once for the whole batch, cells-layout: cell = 32*p + j.
    t8 = tm.reshape(B, CELLS, H).astype(F8)

    in_maps = []
    for c in range(N_CORES):
        sl = slice(c * BS, (c + 1) * BS)
        tcast = t8[sl]                                    # (BS, 4096, 256)
        # cells layout: [s][p][j*H + h], cell = 32p + j
        tc_arr = np.ascontiguousarray(tcast).reshape(BS, 128, NBLK * H)
        # h layout: [s][p][k*CELLS + j*128 + c'] = t[s, cell=32c'+j, h=128k+p]
        x = tcast.reshape(BS, 128, NBLK, 2, 128)          # [s, c', j, k, p]
        th_arr = np.ascontiguousarray(
            x.transpose(0, 4, 3, 2, 1)                    # [s, p, k, j, c']
        ).reshape(BS, 128, 2 * CELLS)
        # qh: [p][2s+k] = qhat[s, 128k+p]
        qh = np.ascontiguousarray(
            qhat[sl].reshape(BS, 2, 128).transpose(2, 0, 1)
        ).reshape(128, BS * 2)
        in_maps.append({
            "th_in": th_arr,
            "tc_in": tc_arr,
            "qh_in": qh,
            "pm_in": np.ascontiguousarray(
                mp[sl].reshape(BS, 128, NBLK).transpose(1, 0, 2)
            ).astype(np.float32).reshape(128, BS * NBLK),
            "nm_in": np.ascontiguousarray(
                mn[sl].reshape(BS, 128, NBLK).transpose(1, 0, 2)
            ).astype(np.float32).reshape(128, BS * NBLK),
        })
    return in_maps, mp, mn


def finish(parts_per_core, mp, mn):
    """parts_per_core: list of (128, 2*BS) arrays -> scalar loss (np.float32)."""
    num = np.zeros(B, np.float32)
    neg = np.zeros(B, np.float32)
    for c in range(N_CORES):
        p = parts_per_core[c]
        for s in range(BS):
            num[c * BS + s] = p[:, 2 * s].sum(dtype=np.float32)
            neg[c * BS + s] = p[:, 2 * s + 1].sum(dtype=np.float32)
    den = num + neg
    with np.errstate(divide="ignore", invalid="ignore", over="ignore"):
        li = -np.log(num / (den + np.float32(EPS)))
    valid = mp.any(axis=(1, 2)) & mn.any(axis=(1, 2))
    n_valid = max(int(valid.sum()), 1)
    loss = np.where(valid, li, np.float32(0.0)).sum(dtype=np.float32) / np.float32(n_valid)
    return np.asarray(loss, dtype=np.float32)


def kernel(pos_query, tmap, mask2d_pos, mask2d_neg):
    in_maps, mp, mn = make_in_maps(pos_query, tmap, mask2d_pos, mask2d_neg)
    nc = get_nc()
    res = run_bass_kernel_spmd(nc, in_maps, list(range(N_CORES)))
    parts_per_core = [res.results[c]["parts"] for c in range(N_CORES)]
    return finish(parts_per_core, mp, mn)


if __name__ == "__main__":
    # Smoke test with random data (no reference).
    rng = np.random.default_rng(0)
    inputs = {
        "pos_query": rng.standard_normal((B, H), dtype=np.float32),
        "tmap": rng.standard_normal((B, S, S, H), dtype=np.float32),
        "mask2d_pos": rng.random((B, S, S)) < 0.05,
        "mask2d_neg": (rng.random((B, S, S)) >= 0.05) & (rng.random((B, S, S)) < 0.35),
    }
    print(kernel(**inputs))
